# revision 21
# baseline (speedup 1.0000x reference)
"""GGNN (gated graph NN) message-passing kernel for Trainium2, 8 NeuronCores.

Model (per reference):
  5 steps of: s_in = state @ in_W.T + in_b ; s_out = state @ out_W.T + out_b
              a_in = A @ s_in ; a_out = A.T @ s_out
              r = sigmoid([a_in, a_out, state] @ r_W.T + r_b)
              z = sigmoid([a_in, a_out, state] @ z_W.T + z_b)
              h = tanh([a_in, a_out, r*state] @ h_W.T + h_b)
              state = (1-z)*state + z*h
  out = tanh(state @ o1_W.T + o1_b) @ o2_W.T + o2_b

Sharding: 1D node parallelism over 8 cores (512 nodes each). Weights
replicated. Per step, each core computes its s_in/s_out shard, all-gathers
them, then computes its row-shard of the aggregations and gates locally.

Precision: the recurrence amplifies operand rounding ~2500x, so bf16 is
far too coarse. Aggregations (the big 4096-contraction GEMMs) use an
exact fp16 hi/lo split of s_in/s_out: A is 0/1 (exact in fp16), so
A@s = A@hi + (1/2048)*A@(2048*(s-hi)) recovers ~22 mantissa bits at
2 passes of full PE rate (2x faster than native fp32 matmul). The lo
part is pre-scaled by 2^11 to dodge fp16 subnormal flushing. All other
GEMMs run native fp32; sigmoid is computed as 0.5+0.5*tanh(x/2) (tanh
LUT is ~50x more accurate than the sigmoid LUT).
"""

import numpy as np

import concourse.bass as bass
import concourse.mybir as mybir
import concourse.tile as tile
from concourse import bacc
from concourse import bass_utils

N = 4096
D = 512
NCORES = 8
NS = N // NCORES          # 512 local nodes per core
NT = NS // 128            # 4 node tiles
DT = D // 128             # 4 feature tiles
CT = 3 * D // 128         # 12 concat-feature tiles
JT = N // 128             # 32 contraction tiles for aggregation
N_STEPS = 5
LO_SCALE = 2048.0         # 2^11: shift s-hi into fp16 normal range

F32 = mybir.dt.float32
F16 = mybir.dt.float16


def _build(n_steps=N_STEPS, use_collective=True):
    nc = bacc.Bacc("TRN2", target_bir_lowering=False, debug=False,
                   enable_asserts=True,
                   num_devices=NCORES if use_collective else 1)

    # ---- per-core external I/O ----
    state_t0 = nc.dram_tensor("state_t0", [D, NS], F32, kind="ExternalInput")
    a_in_rhs = nc.dram_tensor("a_in_rhs", [N, NS], F16, kind="ExternalInput")
    a_out_rhs = nc.dram_tensor("a_out_rhs", [N, NS], F16, kind="ExternalInput")
    in_wt = nc.dram_tensor("in_wt", [D, D], F32, kind="ExternalInput")
    out_wt = nc.dram_tensor("out_wt", [D, D], F32, kind="ExternalInput")
    # gate weights pre-chunked by output tile: [mo, kc*128, 128]
    r_wt = nc.dram_tensor("r_wt", [DT, 3 * D, 128], F32, kind="ExternalInput")
    z_wt = nc.dram_tensor("z_wt", [DT, 3 * D, 128], F32, kind="ExternalInput")
    h_wt = nc.dram_tensor("h_wt", [DT, 3 * D, 128], F32, kind="ExternalInput")
    o1_wt = nc.dram_tensor("o1_wt", [D, D], F32, kind="ExternalInput")
    o2_wt = nc.dram_tensor("o2_wt", [D, D], F32, kind="ExternalInput")
    in_b_row = nc.dram_tensor("in_b_row", [1, D], F32, kind="ExternalInput")
    out_b_row = nc.dram_tensor("out_b_row", [1, D], F32, kind="ExternalInput")
    o2_b_row = nc.dram_tensor("o2_b_row", [1, D], F32, kind="ExternalInput")
    rb_half = nc.dram_tensor("rb_half", [128, DT], F32, kind="ExternalInput")
    zb_half = nc.dram_tensor("zb_half", [128, DT], F32, kind="ExternalInput")
    hb_col = nc.dram_tensor("hb_col", [128, DT], F32, kind="ExternalInput")
    o1b_col = nc.dram_tensor("o1b_col", [128, DT], F32, kind="ExternalInput")
    out_shard = nc.dram_tensor("out_shard", [NS, D], F32, kind="ExternalOutput")

    with tile.TileContext(nc) as tc:
        with (
            tc.tile_pool(name="wpool", bufs=1) as wpool,
            tc.tile_pool(name="state", bufs=2) as spool,
            tc.tile_pool(name="gatew", bufs=3) as gwpool,
            tc.tile_pool(name="work", bufs=2) as work,
            tc.tile_pool(name="sjp", bufs=4) as sjp,
            tc.tile_pool(name="astp", bufs=4) as astp,
            tc.tile_pool(name="att", bufs=2) as att,
            tc.tile_pool(name="gout", bufs=1) as gout,
            tc.tile_pool(name="psum", bufs=8, space="PSUM") as psum,
            tc.tile_pool(name="dram", bufs=2, space="DRAM") as dram,
        ):
            # ---- resident weights / constants ----
            ain_sb = wpool.tile([128, JT, NS], F16, name="ain_sb")
            nc.sync.dma_start(ain_sb[:], a_in_rhs.ap().rearrange("(jt p) r -> p jt r", p=128))
            aout_dram = a_out_rhs.ap().rearrange("(jt p) r -> p jt r", p=128)
            inw_sb = wpool.tile([128, DT, D], F32, name="inw_sb")
            nc.sync.dma_start(inw_sb[:], in_wt.ap().rearrange("(ti p) o -> p ti o", p=128))
            outw_sb = wpool.tile([128, DT, D], F32, name="outw_sb")
            nc.sync.dma_start(outw_sb[:], out_wt.ap().rearrange("(ti p) o -> p ti o", p=128))
            inb_sb = wpool.tile([1, D], F32, name="inb_sb")
            nc.sync.dma_start(inb_sb[:], in_b_row.ap())
            outb_sb = wpool.tile([1, D], F32, name="outb_sb")
            nc.sync.dma_start(outb_sb[:], out_b_row.ap())
            o2b_sb = wpool.tile([1, D], F32, name="o2b_sb")
            nc.sync.dma_start(o2b_sb[:], o2_b_row.ap())
            rbh_sb = wpool.tile([128, DT], F32, name="rbh_sb")
            nc.sync.dma_start(rbh_sb[:], rb_half.ap())
            zbh_sb = wpool.tile([128, DT], F32, name="zbh_sb")
            nc.sync.dma_start(zbh_sb[:], zb_half.ap())
            hb_sb = wpool.tile([128, DT], F32, name="hb_sb")
            nc.sync.dma_start(hb_sb[:], hb_col.ap())
            o1b_sb = wpool.tile([128, DT], F32, name="o1b_sb")
            nc.sync.dma_start(o1b_sb[:], o1b_col.ap())
            ones_sb = wpool.tile([1, 128], F32, name="ones_sb")
            nc.vector.memset(ones_sb[:], 1.0)

            # ---- initial state (feature-major stateT [i, n]) ----
            st = spool.tile([128, DT, NS], F32, name="st")
            nc.sync.dma_start(st[:], state_t0.ap().rearrange("(ti p) n -> p ti n", p=128))

            for step in range(n_steps):
                in_cc_a = dram.tile([NS, 2 * D], F16, name="in_cc_a", tag="icc")
                in_cc_b = dram.tile([NS, 2 * D], F16, name="in_cc_b", tag="icc")
                out_cc_a = dram.tile([N, 2 * D], F16, name="out_cc_a", tag="occ",
                                     addr_space="Shared")
                out_cc_b = dram.tile([N, 2 * D], F16, name="out_cc_b", tag="occ",
                                     addr_space="Shared")

                # ---- GEMM1: s_in / s_out (node-major [n, o]) + bias, f16 hi/lo
                # split; each gathered separately so the s_out gather overlaps
                # the a_in aggregation matmuls
                for w_sb, b_sb, icc, occ_t in ((inw_sb, inb_sb, in_cc_a, out_cc_a),
                                               (outw_sb, outb_sb, in_cc_b, out_cc_b)):
                    # ti-major emission: consecutive PE instructions hit
                    # different PSUM banks (same-bank back-to-back matmuls
                    # serialize on the drain)
                    pss = [psum.tile([128, D], F32, name=f"ps{nt}", tag="ps")
                           for nt in range(NT)]
                    for ti in range(DT):
                        for nt in range(NT):
                            nc.tensor.matmul(pss[nt][:], st[:, ti, nt * 128:(nt + 1) * 128],
                                             w_sb[:, ti, :], start=(ti == 0), stop=False)
                    for nt in range(NT):
                        nc.tensor.matmul(pss[nt][:], ones_sb[:], b_sb[:], start=False, stop=True)
                    for nt in range(NT):
                        hi = work.tile([128, D], F16, name="hi", tag="hi")
                        nc.vector.tensor_copy(hi[:], pss[nt][:])
                        dd = work.tile([128, D], F32, name="dd", tag="dd")
                        nc.vector.tensor_sub(dd[:], pss[nt][:], hi[:])
                        lo = work.tile([128, D], F16, name="lo", tag="lo")
                        nc.vector.tensor_scalar_mul(lo[:], dd[:], LO_SCALE)
                        nc.sync.dma_start(icc[nt * 128:(nt + 1) * 128, 0:D], hi[:])
                        nc.sync.dma_start(icc[nt * 128:(nt + 1) * 128, D:2 * D], lo[:])
                    if use_collective:
                        nc.gpsimd.collective_compute(
                            "AllGather",
                            mybir.AluOpType.bypass,
                            replica_groups=[list(range(NCORES))],
                            ins=[icc.opt()],
                            outs=[occ_t.opt()],
                        )
                    else:
                        # timeline-sim stand-in: own shard only, rest garbage
                        nc.sync.dma_start(occ_t[0:NS, :], icc[:])

                occ_a = out_cc_a.rearrange("(jt p) c -> p jt c", p=128)
                occ_b = out_cc_b.rearrange("(jt p) c -> p jt c", p=128)

                # ---- aggregations: a_inT / a_outT (feature-major [f, r]) ----
                a_inT = att.tile([128, DT, NS], F32, name="a_inT", tag="aT")
                a_outT = att.tile([128, DT, NS], F32, name="a_outT", tag="aT")
                for occ_v, use_ain, dest in ((occ_a, True, a_inT), (occ_b, False, a_outT)):
                    ph = [psum.tile([128, NS], F32, name=f"ph{f}", tag="ps") for f in range(DT)]
                    pl = [psum.tile([128, NS], F32, name=f"pl{f}", tag="ps") for f in range(DT)]
                    for jt in range(JT):
                        sj = sjp.tile([128, 2 * D], F16, name="sj", tag="sj")
                        nc.sync.dma_start(sj[:], occ_v[:, jt, :])
                        if use_ain:
                            a_rhs_tile = ain_sb[:, jt, :]
                        else:
                            ast = astp.tile([128, NS], F16, name="ast", tag="ast")
                            nc.sync.dma_start(ast[:], aout_dram[:, jt, :])
                            a_rhs_tile = ast[:]
                        for f in range(DT):
                            nc.tensor.matmul(ph[f][:], sj[:, f * 128:(f + 1) * 128],
                                             a_rhs_tile, start=(jt == 0), stop=(jt == JT - 1))
                            nc.tensor.matmul(pl[f][:], sj[:, D + f * 128:D + (f + 1) * 128],
                                             a_rhs_tile, start=(jt == 0), stop=(jt == JT - 1))
                    for f in range(DT):
                        tmp = work.tile([128, NS], F32, name="tmph", tag="tmph")
                        nc.vector.tensor_copy(tmp[:], ph[f][:])
                        nc.vector.scalar_tensor_tensor(
                            dest[:, f, :], pl[f][:], 1.0 / LO_SCALE, tmp[:],
                            mybir.AluOpType.mult, mybir.AluOpType.add)

                # ---- gates ----
                def a_rhs(c):
                    if c < DT:
                        return a_inT[:, c, :]
                    if c < 2 * DT:
                        return a_outT[:, c - DT, :]
                    return st[:, c - 2 * DT, :]

                rT = gout.tile([128, DT, NS], F32, name="rT", tag="rT")
                zT = gout.tile([128, DT, NS], F32, name="zT", tag="zT")

                def gate_pair(w_dram, rhs_fn, evict_fn, tilename):
                    # mo processed in pairs with c-major emission: 2-way PSUM
                    # bank interleave avoids same-bank back-to-back matmuls
                    for mo0 in range(0, DT, 2):
                        gw = gwpool.tile([128, CT, 2, 128], F32, name=tilename, tag="gw")
                        for q in range(2):
                            nc.sync.dma_start(
                                gw[:, :, q, :], w_dram.ap()[mo0 + q]
                                .rearrange("(kc p) m -> p kc m", p=128))
                        pss = [psum.tile([128, NS], F32, name=f"psg{q}", tag="ps")
                               for q in range(2)]
                        for c in range(CT):
                            for q in range(2):
                                nc.tensor.matmul(pss[q][:], gw[:, c, q, :],
                                                 rhs_fn(c), start=(c == 0),
                                                 stop=(c == CT - 1))
                        for q in range(2):
                            evict_fn(mo0 + q, pss[q])

                def evict_r(mo, ps):
                    # T_r = tanh(0.5*pre + 0.5*b); r = 0.5 + 0.5*T_r
                    nc.scalar.activation(rT[:, mo, :], ps[:], mybir.ActivationFunctionType.Tanh,
                                         bias=rbh_sb[:, mo:mo + 1], scale=0.5)

                def evict_z(mo, ps):
                    nc.scalar.activation(zT[:, mo, :], ps[:], mybir.ActivationFunctionType.Tanh,
                                         bias=zbh_sb[:, mo:mo + 1], scale=0.5)

                gate_pair(r_wt, a_rhs, evict_r, "gw_r")
                gate_pair(z_wt, a_rhs, evict_z, "gw_z")

                # rs = (0.5 + 0.5*T_r) * state, written in place over rT
                for f in range(DT):
                    rfull = work.tile([128, NS], F32, name="rfull", tag="rfull")
                    nc.vector.tensor_scalar(rfull[:], rT[:, f, :], 0.5, 0.5,
                                            mybir.AluOpType.mult, mybir.AluOpType.add)
                    nc.vector.tensor_mul(rT[:, f, :], rfull[:], st[:, f, :])

                def j_rhs(c):
                    if c < 2 * DT:
                        return a_rhs(c)
                    return rT[:, c - 2 * DT, :]

                st_new = spool.tile([128, DT, NS], F32, name="st")

                def evict_h(mo, ps):
                    hh = work.tile([128, NS], F32, name="hh", tag="hh")
                    nc.scalar.activation(hh[:], ps[:], mybir.ActivationFunctionType.Tanh,
                                         bias=hb_sb[:, mo:mo + 1], scale=1.0)
                    # state' = state + (0.5 + 0.5*T_z)*(h - state)
                    d2 = work.tile([128, NS], F32, name="d2", tag="d2")
                    nc.vector.tensor_sub(d2[:], hh[:], st[:, mo, :])
                    e2 = work.tile([128, NS], F32, name="e2", tag="e2")
                    nc.vector.tensor_mul(e2[:], zT[:, mo, :], d2[:])
                    g2 = work.tile([128, NS], F32, name="g2", tag="g2")
                    nc.vector.tensor_add(g2[:], d2[:], e2[:])
                    nc.vector.scalar_tensor_tensor(
                        st_new[:, mo, :], g2[:], 0.5, st[:, mo, :],
                        mybir.AluOpType.mult, mybir.AluOpType.add)

                gate_pair(h_wt, j_rhs, evict_h, "gw_h")
                st = st_new

            # ---- output MLP ----
            o1w_sb = gwpool.tile([128, DT, D], F32, name="o1w_sb", tag="gw")
            nc.sync.dma_start(o1w_sb[:], o1_wt.ap().rearrange("(ti p) o -> p ti o", p=128))
            o2w_sb = gwpool.tile([128, DT, D], F32, name="o2w_sb", tag="gw")
            nc.sync.dma_start(o2w_sb[:], o2_wt.ap().rearrange("(ti p) o -> p ti o", p=128))

            tT = gout.tile([128, DT, NS], F32, name="tT", tag="rT")
            pst = [psum.tile([128, NS], F32, name=f"ps_t{mo}", tag="ps") for mo in range(DT)]
            for ti in range(DT):
                for mo in range(DT):
                    nc.tensor.matmul(pst[mo][:], o1w_sb[:, ti, mo * 128:(mo + 1) * 128],
                                     st[:, ti, :], start=(ti == 0), stop=(ti == DT - 1))
            for mo in range(DT):
                nc.scalar.activation(tT[:, mo, :], pst[mo][:], mybir.ActivationFunctionType.Tanh,
                                     bias=o1b_sb[:, mo:mo + 1], scale=1.0)

            out_sb = gout.tile([128, NT, D], F32, name="out_sb", tag="zT")
            pso = [psum.tile([128, D], F32, name=f"ps_o{nt}", tag="ps") for nt in range(NT)]
            for c in range(DT):
                for nt in range(NT):
                    nc.tensor.matmul(pso[nt][:], tT[:, c, nt * 128:(nt + 1) * 128],
                                     o2w_sb[:, c, :], start=(c == 0), stop=False)
            for nt in range(NT):
                nc.tensor.matmul(pso[nt][:], ones_sb[:], o2b_sb[:], start=False, stop=True)
                nc.vector.tensor_copy(out_sb[:, nt, :], pso[nt][:])
            nc.sync.dma_start(out_shard.ap().rearrange("(nt p) o -> p nt o", p=128), out_sb[:])

    nc.compile()
    return nc


_NC_CACHE = {}


def _get_nc(n_steps=N_STEPS):
    if n_steps not in _NC_CACHE:
        _NC_CACHE[n_steps] = _build(n_steps)
    return _NC_CACHE[n_steps]


def _prep_in_maps(prop_state, A, in_W, in_b, out_W, out_b, r_W, r_b,
                  z_W, z_b, h_W, h_b, o1_W, o1_b, o2_W, o2_b):
    Af = np.ascontiguousarray(A).astype(np.float32)
    f32 = np.float32

    def rep(x):
        return np.ascontiguousarray(x, dtype=f32)

    def gate_w(W):
        # W.T is [3D, D]; chunk into [mo, 3D, 128] so each output tile's
        # weight block is one contiguous DMA
        return np.ascontiguousarray(W.T.astype(f32).reshape(3 * D, DT, 128).transpose(1, 0, 2))

    shared = {
        "in_wt": rep(in_W.T), "out_wt": rep(out_W.T),
        "r_wt": gate_w(r_W), "z_wt": gate_w(z_W), "h_wt": gate_w(h_W),
        "o1_wt": rep(o1_W.T), "o2_wt": rep(o2_W.T),
        "in_b_row": rep(in_b).reshape(1, D), "out_b_row": rep(out_b).reshape(1, D),
        "o2_b_row": rep(o2_b).reshape(1, D),
        "rb_half": rep(r_b / 2).reshape(DT, 128).T.copy(),
        "zb_half": rep(z_b / 2).reshape(DT, 128).T.copy(),
        "hb_col": rep(h_b).reshape(DT, 128).T.copy(),
        "o1b_col": rep(o1_b).reshape(DT, 128).T.copy(),
    }
    in_maps = []
    for k in range(NCORES):
        rows = slice(k * NS, (k + 1) * NS)
        m = dict(shared)
        m["state_t0"] = np.ascontiguousarray(prop_state[rows].astype(f32).T)
        m["a_in_rhs"] = np.ascontiguousarray(Af[rows, :].T.astype(np.float16))
        m["a_out_rhs"] = np.ascontiguousarray(Af[:, rows].astype(np.float16))
        in_maps.append(m)
    return in_maps


def run(trace=False, **inputs):
    nc = _get_nc()
    in_maps = _prep_in_maps(**inputs)
    res = bass_utils.run_bass_kernel_spmd(
        nc, in_maps, core_ids=list(range(NCORES)), trace=trace)
    out = np.concatenate([res.results[k]["out_shard"] for k in range(NCORES)], axis=0)
    return out, res


def kernel(**inputs) -> np.ndarray:
    out, _ = run(trace=False, **inputs)
    return out


# revision 23
# speedup vs baseline: 1.0366x; 1.0366x over previous
"""GGNN (gated graph NN) message-passing kernel for Trainium2, 8 NeuronCores.

Model (per reference):
  5 steps of: s_in = state @ in_W.T + in_b ; s_out = state @ out_W.T + out_b
              a_in = A @ s_in ; a_out = A.T @ s_out
              r = sigmoid([a_in, a_out, state] @ r_W.T + r_b)
              z = sigmoid([a_in, a_out, state] @ z_W.T + z_b)
              h = tanh([a_in, a_out, r*state] @ h_W.T + h_b)
              state = (1-z)*state + z*h
  out = tanh(state @ o1_W.T + o1_b) @ o2_W.T + o2_b

Sharding: 1D node parallelism over 8 cores (512 nodes each). Weights
replicated. Per step, each core computes its s_in/s_out shard, all-gathers
them, then computes its row-shard of the aggregations and gates locally.

Precision: the recurrence amplifies operand rounding ~2500x, so bf16 is
far too coarse. Aggregations (the big 4096-contraction GEMMs) use an
exact fp16 hi/lo split of s_in/s_out: A is 0/1 (exact in fp16), so
A@s = A@hi + (1/2048)*A@(2048*(s-hi)) recovers ~22 mantissa bits at
2 passes of full PE rate (2x faster than native fp32 matmul). The lo
part is pre-scaled by 2^11 to dodge fp16 subnormal flushing. All other
GEMMs run native fp32; sigmoid is computed as 0.5+0.5*tanh(x/2) (tanh
LUT is ~50x more accurate than the sigmoid LUT).
"""

import numpy as np

import concourse.bass as bass
import concourse.mybir as mybir
import concourse.tile as tile
from concourse import bacc
from concourse import bass_utils

N = 4096
D = 512
NCORES = 8
NS = N // NCORES          # 512 local nodes per core
NT = NS // 128            # 4 node tiles
DT = D // 128             # 4 feature tiles
CT = 3 * D // 128         # 12 concat-feature tiles
JT = N // 128             # 32 contraction tiles for aggregation
N_STEPS = 5
LO_SCALE = 2048.0         # 2^11: shift s-hi into fp16 normal range

F32 = mybir.dt.float32
F16 = mybir.dt.float16


def _build(n_steps=N_STEPS, use_collective=True):
    nc = bacc.Bacc("TRN2", target_bir_lowering=False, debug=False,
                   enable_asserts=True,
                   num_devices=NCORES if use_collective else 1)

    # ---- per-core external I/O ----
    state_t0 = nc.dram_tensor("state_t0", [D, NS], F32, kind="ExternalInput")
    a_in_rhs = nc.dram_tensor("a_in_rhs", [N, NS], F16, kind="ExternalInput")
    a_out_rhs = nc.dram_tensor("a_out_rhs", [N, NS], F16, kind="ExternalInput")
    in_wt = nc.dram_tensor("in_wt", [D, D], F32, kind="ExternalInput")
    out_wt = nc.dram_tensor("out_wt", [D, D], F32, kind="ExternalInput")
    # gate weights pre-chunked by output tile: [mo, kc*128, 128]
    r_wt = nc.dram_tensor("r_wt", [DT, 3 * D, 128], F32, kind="ExternalInput")
    z_wt = nc.dram_tensor("z_wt", [DT, 3 * D, 128], F32, kind="ExternalInput")
    h_wt = nc.dram_tensor("h_wt", [DT, 3 * D, 128], F32, kind="ExternalInput")
    o1_wt = nc.dram_tensor("o1_wt", [D, D], F32, kind="ExternalInput")
    o2_wt = nc.dram_tensor("o2_wt", [D, D], F32, kind="ExternalInput")
    in_b_row = nc.dram_tensor("in_b_row", [1, D], F32, kind="ExternalInput")
    out_b_row = nc.dram_tensor("out_b_row", [1, D], F32, kind="ExternalInput")
    o2_b_row = nc.dram_tensor("o2_b_row", [1, D], F32, kind="ExternalInput")
    rb_half = nc.dram_tensor("rb_half", [128, DT], F32, kind="ExternalInput")
    zb_half = nc.dram_tensor("zb_half", [128, DT], F32, kind="ExternalInput")
    hb_col = nc.dram_tensor("hb_col", [128, DT], F32, kind="ExternalInput")
    o1b_col = nc.dram_tensor("o1b_col", [128, DT], F32, kind="ExternalInput")
    out_shard = nc.dram_tensor("out_shard", [NS, D], F32, kind="ExternalOutput")

    with tile.TileContext(nc) as tc:
        with (
            tc.tile_pool(name="wpool", bufs=1) as wpool,
            tc.tile_pool(name="state", bufs=2) as spool,
            tc.tile_pool(name="gatew", bufs=3) as gwpool,
            tc.tile_pool(name="work", bufs=2) as work,
            tc.tile_pool(name="sjp", bufs=4) as sjp,
            tc.tile_pool(name="astp", bufs=4) as astp,
            tc.tile_pool(name="att", bufs=2) as att,
            tc.tile_pool(name="gout", bufs=1) as gout,
            tc.tile_pool(name="psum", bufs=8, space="PSUM") as psum,
            tc.tile_pool(name="dram", bufs=2, space="DRAM") as dram,
        ):
            # ---- resident weights / constants ----
            ain_sb = wpool.tile([128, JT, NS], F16, name="ain_sb")
            nc.sync.dma_start(ain_sb[:], a_in_rhs.ap().rearrange("(jt p) r -> p jt r", p=128))
            aout_dram = a_out_rhs.ap().rearrange("(jt p) r -> p jt r", p=128)
            inw_sb = wpool.tile([128, DT, D], F32, name="inw_sb")
            nc.sync.dma_start(inw_sb[:], in_wt.ap().rearrange("(ti p) o -> p ti o", p=128))
            outw_sb = wpool.tile([128, DT, D], F32, name="outw_sb")
            nc.sync.dma_start(outw_sb[:], out_wt.ap().rearrange("(ti p) o -> p ti o", p=128))
            inb_sb = wpool.tile([1, D], F32, name="inb_sb")
            nc.sync.dma_start(inb_sb[:], in_b_row.ap())
            outb_sb = wpool.tile([1, D], F32, name="outb_sb")
            nc.sync.dma_start(outb_sb[:], out_b_row.ap())
            o2b_sb = wpool.tile([1, D], F32, name="o2b_sb")
            nc.sync.dma_start(o2b_sb[:], o2_b_row.ap())
            rbh_sb = wpool.tile([128, DT], F32, name="rbh_sb")
            nc.sync.dma_start(rbh_sb[:], rb_half.ap())
            zbh_sb = wpool.tile([128, DT], F32, name="zbh_sb")
            nc.sync.dma_start(zbh_sb[:], zb_half.ap())
            hb_sb = wpool.tile([128, DT], F32, name="hb_sb")
            nc.sync.dma_start(hb_sb[:], hb_col.ap())
            o1b_sb = wpool.tile([128, DT], F32, name="o1b_sb")
            nc.sync.dma_start(o1b_sb[:], o1b_col.ap())
            ones_sb = wpool.tile([1, 128], F32, name="ones_sb")
            nc.vector.memset(ones_sb[:], 1.0)

            # ---- initial state (feature-major stateT [i, n]) ----
            st = spool.tile([128, DT, NS], F32, name="st")
            nc.sync.dma_start(st[:], state_t0.ap().rearrange("(ti p) n -> p ti n", p=128))

            for step in range(n_steps):
                in_cc_a = dram.tile([NS, 2 * D], F16, name="in_cc_a", tag="icc")
                in_cc_b = dram.tile([NS, 2 * D], F16, name="in_cc_b", tag="icc")
                # s_in gathered per 128-row block (4 small AllGathers) so the
                # first aggregation matmuls can start while GEMM1 is still
                # running; s_out as one gather (it hides under a_in compute).
                out_ccs = [dram.tile([8 * 128, 2 * D], F16, name=f"occ_a{nt}",
                                     tag="occa", addr_space="Shared")
                           for nt in range(NT)]
                out_cc_b = dram.tile([N, 2 * D], F16, name="out_cc_b", tag="occ",
                                     addr_space="Shared")

                # ---- GEMM1: s_in / s_out (node-major [n, o]) + bias, f16 hi/lo
                for w_sb, b_sb, icc, blockwise in ((inw_sb, inb_sb, in_cc_a, True),
                                                   (outw_sb, outb_sb, in_cc_b, False)):
                    for nt in range(NT):
                        ps = psum.tile([128, D], F32, name="ps", tag="ps")
                        for ti in range(DT):
                            nc.tensor.matmul(ps[:], st[:, ti, nt * 128:(nt + 1) * 128],
                                             w_sb[:, ti, :], start=(ti == 0), stop=False)
                        nc.tensor.matmul(ps[:], ones_sb[:], b_sb[:], start=False, stop=True)
                        hi = work.tile([128, D], F16, name="hi", tag="hi")
                        nc.vector.tensor_copy(hi[:], ps[:])
                        dd = work.tile([128, D], F32, name="dd", tag="dd")
                        nc.vector.tensor_sub(dd[:], ps[:], hi[:])
                        lo = work.tile([128, D], F16, name="lo", tag="lo")
                        nc.vector.tensor_scalar_mul(lo[:], dd[:], LO_SCALE)
                        nc.sync.dma_start(icc[nt * 128:(nt + 1) * 128, 0:D], hi[:])
                        nc.sync.dma_start(icc[nt * 128:(nt + 1) * 128, D:2 * D], lo[:])
                        if blockwise and use_collective:
                            nc.gpsimd.collective_compute(
                                "AllGather",
                                mybir.AluOpType.bypass,
                                replica_groups=[list(range(NCORES))],
                                ins=[icc[nt * 128:(nt + 1) * 128, :].opt()],
                                outs=[out_ccs[nt].opt()],
                            )
                    if not blockwise:
                        if use_collective:
                            nc.gpsimd.collective_compute(
                                "AllGather",
                                mybir.AluOpType.bypass,
                                replica_groups=[list(range(NCORES))],
                                ins=[icc.opt()],
                                outs=[out_cc_b.opt()],
                            )
                        else:
                            nc.sync.dma_start(out_cc_b[0:NS, :], icc[:])
                    elif not use_collective:
                        for nt in range(NT):
                            nc.sync.dma_start(out_ccs[nt][0:128, :],
                                              in_cc_a[nt * 128:(nt + 1) * 128, :])

                occ_b = out_cc_b.rearrange("(jt p) c -> p jt c", p=128)

                # ---- aggregations: a_inT / a_outT (feature-major [f, r]) ----
                a_inT = att.tile([128, DT, NS], F32, name="a_inT", tag="aT")
                a_outT = att.tile([128, DT, NS], F32, name="a_outT", tag="aT")

                # a_in: consume gather blocks in arrival order; block b core c
                # holds nodes j-tile c*NT + b
                ph = [psum.tile([128, NS], F32, name=f"ph{f}", tag="ps") for f in range(DT)]
                pl = [psum.tile([128, NS], F32, name=f"pl{f}", tag="ps") for f in range(DT)]
                for b in range(NT):
                    occ_bv = out_ccs[b].rearrange("(cc p) c -> p cc c", p=128)
                    for cc in range(NCORES):
                        jt = cc * NT + b
                        sj = sjp.tile([128, 2 * D], F16, name="sj", tag="sj")
                        nc.sync.dma_start(sj[:], occ_bv[:, cc, :])
                        first = (b == 0 and cc == 0)
                        last = (b == NT - 1 and cc == NCORES - 1)
                        for f in range(DT):
                            nc.tensor.matmul(ph[f][:], sj[:, f * 128:(f + 1) * 128],
                                             ain_sb[:, jt, :], start=first, stop=last)
                            nc.tensor.matmul(pl[f][:], sj[:, D + f * 128:D + (f + 1) * 128],
                                             ain_sb[:, jt, :], start=first, stop=last)
                for f in range(DT):
                    tmp = work.tile([128, NS], F32, name="tmph", tag="tmph")
                    nc.vector.tensor_copy(tmp[:], ph[f][:])
                    nc.vector.scalar_tensor_tensor(
                        a_inT[:, f, :], pl[f][:], 1.0 / LO_SCALE, tmp[:],
                        mybir.AluOpType.mult, mybir.AluOpType.add)

                # a_out
                ph = [psum.tile([128, NS], F32, name=f"qh{f}", tag="ps") for f in range(DT)]
                pl = [psum.tile([128, NS], F32, name=f"ql{f}", tag="ps") for f in range(DT)]
                for jt in range(JT):
                    sj = sjp.tile([128, 2 * D], F16, name="sj", tag="sj")
                    nc.sync.dma_start(sj[:], occ_b[:, jt, :])
                    ast = astp.tile([128, NS], F16, name="ast", tag="ast")
                    nc.sync.dma_start(ast[:], aout_dram[:, jt, :])
                    for f in range(DT):
                        nc.tensor.matmul(ph[f][:], sj[:, f * 128:(f + 1) * 128],
                                         ast[:], start=(jt == 0), stop=(jt == JT - 1))
                        nc.tensor.matmul(pl[f][:], sj[:, D + f * 128:D + (f + 1) * 128],
                                         ast[:], start=(jt == 0), stop=(jt == JT - 1))
                for f in range(DT):
                    tmp = work.tile([128, NS], F32, name="tmph", tag="tmph")
                    nc.vector.tensor_copy(tmp[:], ph[f][:])
                    nc.vector.scalar_tensor_tensor(
                        a_outT[:, f, :], pl[f][:], 1.0 / LO_SCALE, tmp[:],
                        mybir.AluOpType.mult, mybir.AluOpType.add)

                # ---- gates ----
                def a_rhs(c):
                    if c < DT:
                        return a_inT[:, c, :]
                    if c < 2 * DT:
                        return a_outT[:, c - DT, :]
                    return st[:, c - 2 * DT, :]

                rT = gout.tile([128, DT, NS], F32, name="rT", tag="rT")
                zT = gout.tile([128, DT, NS], F32, name="zT", tag="zT")

                def gate_pair(w_dram, rhs_fn, evict_fn, tilename):
                    for mo in range(DT):
                        gw = gwpool.tile([128, CT, 128], F32, name=tilename, tag="gw")
                        nc.sync.dma_start(
                            gw[:], w_dram.ap()[mo].rearrange("(kc p) m -> p kc m", p=128))
                        ps = psum.tile([128, NS], F32, name="psg", tag="ps")
                        for c in range(CT):
                            nc.tensor.matmul(ps[:], gw[:, c, :], rhs_fn(c),
                                             start=(c == 0), stop=(c == CT - 1))
                        evict_fn(mo, ps)

                def evict_r(mo, ps):
                    # T_r = tanh(0.5*pre + 0.5*b); r = 0.5 + 0.5*T_r
                    nc.scalar.activation(rT[:, mo, :], ps[:], mybir.ActivationFunctionType.Tanh,
                                         bias=rbh_sb[:, mo:mo + 1], scale=0.5)

                def evict_z(mo, ps):
                    nc.scalar.activation(zT[:, mo, :], ps[:], mybir.ActivationFunctionType.Tanh,
                                         bias=zbh_sb[:, mo:mo + 1], scale=0.5)

                gate_pair(r_wt, a_rhs, evict_r, "gw_r")
                gate_pair(z_wt, a_rhs, evict_z, "gw_z")

                # rs = (0.5 + 0.5*T_r) * state, written in place over rT
                for f in range(DT):
                    rfull = work.tile([128, NS], F32, name="rfull", tag="rfull")
                    nc.vector.tensor_scalar(rfull[:], rT[:, f, :], 0.5, 0.5,
                                            mybir.AluOpType.mult, mybir.AluOpType.add)
                    nc.vector.tensor_mul(rT[:, f, :], rfull[:], st[:, f, :])

                def j_rhs(c):
                    if c < 2 * DT:
                        return a_rhs(c)
                    return rT[:, c - 2 * DT, :]

                st_new = spool.tile([128, DT, NS], F32, name="st")

                def evict_h(mo, ps):
                    hh = work.tile([128, NS], F32, name="hh", tag="hh")
                    nc.scalar.activation(hh[:], ps[:], mybir.ActivationFunctionType.Tanh,
                                         bias=hb_sb[:, mo:mo + 1], scale=1.0)
                    # state' = state + (0.5 + 0.5*T_z)*(h - state)
                    d2 = work.tile([128, NS], F32, name="d2", tag="d2")
                    nc.vector.tensor_sub(d2[:], hh[:], st[:, mo, :])
                    e2 = work.tile([128, NS], F32, name="e2", tag="e2")
                    nc.vector.tensor_mul(e2[:], zT[:, mo, :], d2[:])
                    g2 = work.tile([128, NS], F32, name="g2", tag="g2")
                    nc.vector.tensor_add(g2[:], d2[:], e2[:])
                    nc.vector.scalar_tensor_tensor(
                        st_new[:, mo, :], g2[:], 0.5, st[:, mo, :],
                        mybir.AluOpType.mult, mybir.AluOpType.add)

                gate_pair(h_wt, j_rhs, evict_h, "gw_h")
                st = st_new

            # ---- output MLP ----
            o1w_sb = gwpool.tile([128, DT, D], F32, name="o1w_sb", tag="gw")
            nc.sync.dma_start(o1w_sb[:], o1_wt.ap().rearrange("(ti p) o -> p ti o", p=128))
            o2w_sb = gwpool.tile([128, DT, D], F32, name="o2w_sb", tag="gw")
            nc.sync.dma_start(o2w_sb[:], o2_wt.ap().rearrange("(ti p) o -> p ti o", p=128))

            tT = gout.tile([128, DT, NS], F32, name="tT", tag="rT")
            pst = [psum.tile([128, NS], F32, name=f"ps_t{mo}", tag="ps") for mo in range(DT)]
            for ti in range(DT):
                for mo in range(DT):
                    nc.tensor.matmul(pst[mo][:], o1w_sb[:, ti, mo * 128:(mo + 1) * 128],
                                     st[:, ti, :], start=(ti == 0), stop=(ti == DT - 1))
            for mo in range(DT):
                nc.scalar.activation(tT[:, mo, :], pst[mo][:], mybir.ActivationFunctionType.Tanh,
                                     bias=o1b_sb[:, mo:mo + 1], scale=1.0)

            out_sb = gout.tile([128, NT, D], F32, name="out_sb", tag="zT")
            pso = [psum.tile([128, D], F32, name=f"ps_o{nt}", tag="ps") for nt in range(NT)]
            for c in range(DT):
                for nt in range(NT):
                    nc.tensor.matmul(pso[nt][:], tT[:, c, nt * 128:(nt + 1) * 128],
                                     o2w_sb[:, c, :], start=(c == 0), stop=False)
            for nt in range(NT):
                nc.tensor.matmul(pso[nt][:], ones_sb[:], o2b_sb[:], start=False, stop=True)
                nc.vector.tensor_copy(out_sb[:, nt, :], pso[nt][:])
            nc.sync.dma_start(out_shard.ap().rearrange("(nt p) o -> p nt o", p=128), out_sb[:])

    nc.compile()
    return nc


_NC_CACHE = {}


def _get_nc(n_steps=N_STEPS):
    if n_steps not in _NC_CACHE:
        _NC_CACHE[n_steps] = _build(n_steps)
    return _NC_CACHE[n_steps]


def _prep_in_maps(prop_state, A, in_W, in_b, out_W, out_b, r_W, r_b,
                  z_W, z_b, h_W, h_b, o1_W, o1_b, o2_W, o2_b):
    Af = np.ascontiguousarray(A).astype(np.float32)
    f32 = np.float32

    def rep(x):
        return np.ascontiguousarray(x, dtype=f32)

    def gate_w(W):
        # W.T is [3D, D]; chunk into [mo, 3D, 128] so each output tile's
        # weight block is one contiguous DMA
        return np.ascontiguousarray(W.T.astype(f32).reshape(3 * D, DT, 128).transpose(1, 0, 2))

    shared = {
        "in_wt": rep(in_W.T), "out_wt": rep(out_W.T),
        "r_wt": gate_w(r_W), "z_wt": gate_w(z_W), "h_wt": gate_w(h_W),
        "o1_wt": rep(o1_W.T), "o2_wt": rep(o2_W.T),
        "in_b_row": rep(in_b).reshape(1, D), "out_b_row": rep(out_b).reshape(1, D),
        "o2_b_row": rep(o2_b).reshape(1, D),
        "rb_half": rep(r_b / 2).reshape(DT, 128).T.copy(),
        "zb_half": rep(z_b / 2).reshape(DT, 128).T.copy(),
        "hb_col": rep(h_b).reshape(DT, 128).T.copy(),
        "o1b_col": rep(o1_b).reshape(DT, 128).T.copy(),
    }
    in_maps = []
    for k in range(NCORES):
        rows = slice(k * NS, (k + 1) * NS)
        m = dict(shared)
        m["state_t0"] = np.ascontiguousarray(prop_state[rows].astype(f32).T)
        m["a_in_rhs"] = np.ascontiguousarray(Af[rows, :].T.astype(np.float16))
        m["a_out_rhs"] = np.ascontiguousarray(Af[:, rows].astype(np.float16))
        in_maps.append(m)
    return in_maps


def run(trace=False, **inputs):
    nc = _get_nc()
    in_maps = _prep_in_maps(**inputs)
    res = bass_utils.run_bass_kernel_spmd(
        nc, in_maps, core_ids=list(range(NCORES)), trace=trace)
    out = np.concatenate([res.results[k]["out_shard"] for k in range(NCORES)], axis=0)
    return out, res


def kernel(**inputs) -> np.ndarray:
    out, _ = run(trace=False, **inputs)
    return out


# revision 24
# speedup vs baseline: 2.1027x; 2.0285x over previous
"""GGNN (gated graph NN) message-passing kernel for Trainium2, 8 NeuronCores.

Model (per reference):
  5 steps of: s_in = state @ in_W.T + in_b ; s_out = state @ out_W.T + out_b
              a_in = A @ s_in ; a_out = A.T @ s_out
              r = sigmoid([a_in, a_out, state] @ r_W.T + r_b)
              z = sigmoid([a_in, a_out, state] @ z_W.T + z_b)
              h = tanh([a_in, a_out, r*state] @ h_W.T + h_b)
              state = (1-z)*state + z*h
  out = tanh(state @ o1_W.T + o1_b) @ o2_W.T + o2_b

Sharding: 1D node parallelism over 8 cores (512 nodes each). Weights
replicated. Per step, each core computes its s_in/s_out shard, all-gathers
them, then computes its row-shard of the aggregations and gates locally.

Precision: the recurrence amplifies operand rounding ~2500x, so bf16 is
far too coarse. Aggregations (the big 4096-contraction GEMMs) use an
exact fp16 hi/lo split of s_in/s_out: A is 0/1 (exact in fp16), so
A@s = A@hi + (1/2048)*A@(2048*(s-hi)) recovers ~22 mantissa bits at
2 passes of full PE rate (2x faster than native fp32 matmul). The lo
part is pre-scaled by 2^11 to dodge fp16 subnormal flushing. All other
GEMMs run native fp32; sigmoid is computed as 0.5+0.5*tanh(x/2) (tanh
LUT is ~50x more accurate than the sigmoid LUT).
"""

import numpy as np

import concourse.bass as bass
import concourse.mybir as mybir
import concourse.tile as tile
from concourse import bacc
from concourse import bass_utils

N = 4096
D = 512
NCORES = 8
NS = N // NCORES          # 512 local nodes per core
NT = NS // 128            # 4 node tiles
DT = D // 128             # 4 feature tiles
CT = 3 * D // 128         # 12 concat-feature tiles
JT = N // 128             # 32 contraction tiles for aggregation
N_STEPS = 5
LO_SCALE = 2048.0         # 2^11: shift s-hi into fp16 normal range

F32 = mybir.dt.float32
F16 = mybir.dt.float16


def _build(n_steps=N_STEPS, use_collective=True):
    nc = bacc.Bacc("TRN2", target_bir_lowering=False, debug=False,
                   enable_asserts=True,
                   num_devices=NCORES if use_collective else 1)

    # ---- per-core external I/O ----
    state_t0 = nc.dram_tensor("state_t0", [D, NS], F32, kind="ExternalInput")
    a_in_rhs = nc.dram_tensor("a_in_rhs", [N, NS], F16, kind="ExternalInput")
    a_out_rhs = nc.dram_tensor("a_out_rhs", [N, NS], F16, kind="ExternalInput")
    in_wt = nc.dram_tensor("in_wt", [D, D], F32, kind="ExternalInput")
    out_wt = nc.dram_tensor("out_wt", [D, D], F32, kind="ExternalInput")
    # gate weights pre-chunked by output tile: [mo, kc*128, 128]
    r_wt = nc.dram_tensor("r_wt", [DT, 3 * D, 128], F32, kind="ExternalInput")
    z_wt = nc.dram_tensor("z_wt", [DT, 3 * D, 128], F32, kind="ExternalInput")
    h_wt = nc.dram_tensor("h_wt", [DT, 3 * D, 128], F32, kind="ExternalInput")
    o1_wt = nc.dram_tensor("o1_wt", [D, D], F32, kind="ExternalInput")
    o2_wt = nc.dram_tensor("o2_wt", [D, D], F32, kind="ExternalInput")
    in_b_row = nc.dram_tensor("in_b_row", [1, D], F32, kind="ExternalInput")
    out_b_row = nc.dram_tensor("out_b_row", [1, D], F32, kind="ExternalInput")
    o2_b_row = nc.dram_tensor("o2_b_row", [1, D], F32, kind="ExternalInput")
    rb_half = nc.dram_tensor("rb_half", [128, DT], F32, kind="ExternalInput")
    zb_half = nc.dram_tensor("zb_half", [128, DT], F32, kind="ExternalInput")
    hb_col = nc.dram_tensor("hb_col", [128, DT], F32, kind="ExternalInput")
    o1b_col = nc.dram_tensor("o1b_col", [128, DT], F32, kind="ExternalInput")
    out_shard = nc.dram_tensor("out_shard", [NS, D], F32, kind="ExternalOutput")

    with tile.TileContext(nc) as tc:
        with (
            tc.tile_pool(name="wpool", bufs=1) as wpool,
            tc.tile_pool(name="state", bufs=2) as spool,
            tc.tile_pool(name="gatew", bufs=3) as gwpool,
            tc.tile_pool(name="work", bufs=2) as work,
            tc.tile_pool(name="sjp", bufs=10) as sjp,
            tc.tile_pool(name="astp", bufs=10) as astp,
            tc.tile_pool(name="att", bufs=2) as att,
            tc.tile_pool(name="gout", bufs=1) as gout,
            tc.tile_pool(name="psum", bufs=8, space="PSUM") as psum,
            tc.tile_pool(name="dram", bufs=2, space="DRAM") as dram,
        ):
            # ---- resident weights / constants ----
            ain_sb = wpool.tile([128, JT, NS], F16, name="ain_sb")
            nc.sync.dma_start(ain_sb[:], a_in_rhs.ap().rearrange("(jt p) r -> p jt r", p=128))
            aout_dram = a_out_rhs.ap().rearrange("(jt p) r -> p jt r", p=128)
            inw_sb = wpool.tile([128, DT, D], F32, name="inw_sb")
            nc.sync.dma_start(inw_sb[:], in_wt.ap().rearrange("(ti p) o -> p ti o", p=128))
            outw_sb = wpool.tile([128, DT, D], F32, name="outw_sb")
            nc.sync.dma_start(outw_sb[:], out_wt.ap().rearrange("(ti p) o -> p ti o", p=128))
            inb_sb = wpool.tile([1, D], F32, name="inb_sb")
            nc.sync.dma_start(inb_sb[:], in_b_row.ap())
            outb_sb = wpool.tile([1, D], F32, name="outb_sb")
            nc.sync.dma_start(outb_sb[:], out_b_row.ap())
            o2b_sb = wpool.tile([1, D], F32, name="o2b_sb")
            nc.sync.dma_start(o2b_sb[:], o2_b_row.ap())
            rbh_sb = wpool.tile([128, DT], F32, name="rbh_sb")
            nc.sync.dma_start(rbh_sb[:], rb_half.ap())
            zbh_sb = wpool.tile([128, DT], F32, name="zbh_sb")
            nc.sync.dma_start(zbh_sb[:], zb_half.ap())
            hb_sb = wpool.tile([128, DT], F32, name="hb_sb")
            nc.sync.dma_start(hb_sb[:], hb_col.ap())
            o1b_sb = wpool.tile([128, DT], F32, name="o1b_sb")
            nc.sync.dma_start(o1b_sb[:], o1b_col.ap())
            ones_sb = wpool.tile([1, 128], F32, name="ones_sb")
            nc.vector.memset(ones_sb[:], 1.0)

            # ---- initial state (feature-major stateT [i, n]) ----
            st = spool.tile([128, DT, NS], F32, name="st")
            nc.sync.dma_start(st[:], state_t0.ap().rearrange("(ti p) n -> p ti n", p=128))

            for step in range(n_steps):
                in_cc_a = dram.tile([NS, 2 * D], F16, name="in_cc_a", tag="icc")
                in_cc_b = dram.tile([NS, 2 * D], F16, name="in_cc_b", tag="icc")
                # s_in gathered per 128-row block (4 small AllGathers) so the
                # first aggregation matmuls can start while GEMM1 is still
                # running; s_out as one gather (it hides under a_in compute).
                out_ccs = [dram.tile([8 * 128, 2 * D], F16, name=f"occ_a{nt}",
                                     tag="occa", addr_space="Shared")
                           for nt in range(NT)]
                out_cc_b = dram.tile([N, 2 * D], F16, name="out_cc_b", tag="occ",
                                     addr_space="Shared")

                # ---- GEMM1: s_in / s_out (node-major [n, o]) + bias, f16 hi/lo
                for w_sb, b_sb, icc, blockwise in ((inw_sb, inb_sb, in_cc_a, True),
                                                   (outw_sb, outb_sb, in_cc_b, False)):
                    for nt in range(NT):
                        ps = psum.tile([128, D], F32, name="ps", tag="ps")
                        for ti in range(DT):
                            nc.tensor.matmul(ps[:], st[:, ti, nt * 128:(nt + 1) * 128],
                                             w_sb[:, ti, :], start=(ti == 0), stop=False)
                        nc.tensor.matmul(ps[:], ones_sb[:], b_sb[:], start=False, stop=True)
                        hi = work.tile([128, D], F16, name="hi", tag="hi")
                        nc.vector.tensor_copy(hi[:], ps[:])
                        dd = work.tile([128, D], F32, name="dd", tag="dd")
                        nc.vector.tensor_sub(dd[:], ps[:], hi[:])
                        lo = work.tile([128, D], F16, name="lo", tag="lo")
                        nc.vector.tensor_scalar_mul(lo[:], dd[:], LO_SCALE)
                        nc.sync.dma_start(icc[nt * 128:(nt + 1) * 128, 0:D], hi[:])
                        nc.sync.dma_start(icc[nt * 128:(nt + 1) * 128, D:2 * D], lo[:])
                        if blockwise and use_collective:
                            nc.gpsimd.collective_compute(
                                "AllGather",
                                mybir.AluOpType.bypass,
                                replica_groups=[list(range(NCORES))],
                                ins=[icc[nt * 128:(nt + 1) * 128, :].opt()],
                                outs=[out_ccs[nt].opt()],
                            )
                    if not blockwise:
                        if use_collective:
                            nc.gpsimd.collective_compute(
                                "AllGather",
                                mybir.AluOpType.bypass,
                                replica_groups=[list(range(NCORES))],
                                ins=[icc.opt()],
                                outs=[out_cc_b.opt()],
                            )
                        else:
                            nc.sync.dma_start(out_cc_b[0:NS, :], icc[:])
                    elif not use_collective:
                        for nt in range(NT):
                            nc.sync.dma_start(out_ccs[nt][0:128, :],
                                              in_cc_a[nt * 128:(nt + 1) * 128, :])

                occ_b = out_cc_b.rearrange("(jt p) c -> p jt c", p=128)

                # ---- aggregations: a_inT / a_outT (feature-major [f, r]) ----
                a_inT = att.tile([128, DT, NS], F32, name="a_inT", tag="aT")
                a_outT = att.tile([128, DT, NS], F32, name="a_outT", tag="aT")

                # a_in: consume gather blocks in arrival order; block b core c
                # holds nodes j-tile c*NT + b
                ph = [psum.tile([128, NS], F32, name=f"ph{f}", tag="ps") for f in range(DT)]
                pl = [psum.tile([128, NS], F32, name=f"pl{f}", tag="ps") for f in range(DT)]
                for b in range(NT):
                    occ_bv = out_ccs[b].rearrange("(cc p) c -> p cc c", p=128)
                    for cc in range(NCORES):
                        jt = cc * NT + b
                        sj = sjp.tile([128, 2 * D], F16, name="sj", tag="sj")
                        nc.sync.dma_start(sj[:], occ_bv[:, cc, :])
                        first = (b == 0 and cc == 0)
                        last = (b == NT - 1 and cc == NCORES - 1)
                        for f in range(DT):
                            nc.tensor.matmul(ph[f][:], sj[:, f * 128:(f + 1) * 128],
                                             ain_sb[:, jt, :], start=first, stop=last)
                            nc.tensor.matmul(pl[f][:], sj[:, D + f * 128:D + (f + 1) * 128],
                                             ain_sb[:, jt, :], start=first, stop=last)
                for f in range(DT):
                    tmp = work.tile([128, NS], F32, name="tmph", tag="tmph")
                    nc.vector.tensor_copy(tmp[:], ph[f][:])
                    nc.vector.scalar_tensor_tensor(
                        a_inT[:, f, :], pl[f][:], 1.0 / LO_SCALE, tmp[:],
                        mybir.AluOpType.mult, mybir.AluOpType.add)

                # a_out
                ph = [psum.tile([128, NS], F32, name=f"qh{f}", tag="ps") for f in range(DT)]
                pl = [psum.tile([128, NS], F32, name=f"ql{f}", tag="ps") for f in range(DT)]
                for jt in range(JT):
                    sj = sjp.tile([128, 2 * D], F16, name="sj", tag="sj")
                    nc.sync.dma_start(sj[:], occ_b[:, jt, :])
                    ast = astp.tile([128, NS], F16, name="ast", tag="ast")
                    nc.gpsimd.dma_start(ast[:], aout_dram[:, jt, :])
                    for f in range(DT):
                        nc.tensor.matmul(ph[f][:], sj[:, f * 128:(f + 1) * 128],
                                         ast[:], start=(jt == 0), stop=(jt == JT - 1))
                        nc.tensor.matmul(pl[f][:], sj[:, D + f * 128:D + (f + 1) * 128],
                                         ast[:], start=(jt == 0), stop=(jt == JT - 1))
                for f in range(DT):
                    tmp = work.tile([128, NS], F32, name="tmph", tag="tmph")
                    nc.vector.tensor_copy(tmp[:], ph[f][:])
                    nc.vector.scalar_tensor_tensor(
                        a_outT[:, f, :], pl[f][:], 1.0 / LO_SCALE, tmp[:],
                        mybir.AluOpType.mult, mybir.AluOpType.add)

                # ---- gates ----
                def a_rhs(c):
                    if c < DT:
                        return a_inT[:, c, :]
                    if c < 2 * DT:
                        return a_outT[:, c - DT, :]
                    return st[:, c - 2 * DT, :]

                rT = gout.tile([128, DT, NS], F32, name="rT", tag="rT")
                zT = gout.tile([128, DT, NS], F32, name="zT", tag="zT")

                def gate_pair(w_dram, rhs_fn, evict_fn, tilename):
                    for mo in range(DT):
                        gw = gwpool.tile([128, CT, 128], F32, name=tilename, tag="gw")
                        nc.gpsimd.dma_start(
                            gw[:], w_dram.ap()[mo].rearrange("(kc p) m -> p kc m", p=128))
                        ps = psum.tile([128, NS], F32, name="psg", tag="ps")
                        for c in range(CT):
                            nc.tensor.matmul(ps[:], gw[:, c, :], rhs_fn(c),
                                             start=(c == 0), stop=(c == CT - 1))
                        evict_fn(mo, ps)

                def evict_r(mo, ps):
                    # T_r = tanh(0.5*pre + 0.5*b); r = 0.5 + 0.5*T_r
                    nc.scalar.activation(rT[:, mo, :], ps[:], mybir.ActivationFunctionType.Tanh,
                                         bias=rbh_sb[:, mo:mo + 1], scale=0.5)

                def evict_z(mo, ps):
                    nc.scalar.activation(zT[:, mo, :], ps[:], mybir.ActivationFunctionType.Tanh,
                                         bias=zbh_sb[:, mo:mo + 1], scale=0.5)

                gate_pair(r_wt, a_rhs, evict_r, "gw_r")
                gate_pair(z_wt, a_rhs, evict_z, "gw_z")

                # rs = (0.5 + 0.5*T_r) * state, written in place over rT
                for f in range(DT):
                    rfull = work.tile([128, NS], F32, name="rfull", tag="rfull")
                    nc.vector.tensor_scalar(rfull[:], rT[:, f, :], 0.5, 0.5,
                                            mybir.AluOpType.mult, mybir.AluOpType.add)
                    nc.vector.tensor_mul(rT[:, f, :], rfull[:], st[:, f, :])

                def j_rhs(c):
                    if c < 2 * DT:
                        return a_rhs(c)
                    return rT[:, c - 2 * DT, :]

                st_new = spool.tile([128, DT, NS], F32, name="st")

                def evict_h(mo, ps):
                    hh = work.tile([128, NS], F32, name="hh", tag="hh")
                    nc.scalar.activation(hh[:], ps[:], mybir.ActivationFunctionType.Tanh,
                                         bias=hb_sb[:, mo:mo + 1], scale=1.0)
                    # state' = state + (0.5 + 0.5*T_z)*(h - state)
                    d2 = work.tile([128, NS], F32, name="d2", tag="d2")
                    nc.vector.tensor_sub(d2[:], hh[:], st[:, mo, :])
                    e2 = work.tile([128, NS], F32, name="e2", tag="e2")
                    nc.vector.tensor_mul(e2[:], zT[:, mo, :], d2[:])
                    g2 = work.tile([128, NS], F32, name="g2", tag="g2")
                    nc.vector.tensor_add(g2[:], d2[:], e2[:])
                    nc.vector.scalar_tensor_tensor(
                        st_new[:, mo, :], g2[:], 0.5, st[:, mo, :],
                        mybir.AluOpType.mult, mybir.AluOpType.add)

                gate_pair(h_wt, j_rhs, evict_h, "gw_h")
                st = st_new

            # ---- output MLP ----
            o1w_sb = gwpool.tile([128, DT, D], F32, name="o1w_sb", tag="gw")
            nc.sync.dma_start(o1w_sb[:], o1_wt.ap().rearrange("(ti p) o -> p ti o", p=128))
            o2w_sb = gwpool.tile([128, DT, D], F32, name="o2w_sb", tag="gw")
            nc.sync.dma_start(o2w_sb[:], o2_wt.ap().rearrange("(ti p) o -> p ti o", p=128))

            tT = gout.tile([128, DT, NS], F32, name="tT", tag="rT")
            pst = [psum.tile([128, NS], F32, name=f"ps_t{mo}", tag="ps") for mo in range(DT)]
            for ti in range(DT):
                for mo in range(DT):
                    nc.tensor.matmul(pst[mo][:], o1w_sb[:, ti, mo * 128:(mo + 1) * 128],
                                     st[:, ti, :], start=(ti == 0), stop=(ti == DT - 1))
            for mo in range(DT):
                nc.scalar.activation(tT[:, mo, :], pst[mo][:], mybir.ActivationFunctionType.Tanh,
                                     bias=o1b_sb[:, mo:mo + 1], scale=1.0)

            out_sb = gout.tile([128, NT, D], F32, name="out_sb", tag="zT")
            pso = [psum.tile([128, D], F32, name=f"ps_o{nt}", tag="ps") for nt in range(NT)]
            for c in range(DT):
                for nt in range(NT):
                    nc.tensor.matmul(pso[nt][:], tT[:, c, nt * 128:(nt + 1) * 128],
                                     o2w_sb[:, c, :], start=(c == 0), stop=False)
            for nt in range(NT):
                nc.tensor.matmul(pso[nt][:], ones_sb[:], o2b_sb[:], start=False, stop=True)
                nc.vector.tensor_copy(out_sb[:, nt, :], pso[nt][:])
            nc.sync.dma_start(out_shard.ap().rearrange("(nt p) o -> p nt o", p=128), out_sb[:])

    nc.compile()
    return nc


_NC_CACHE = {}


def _get_nc(n_steps=N_STEPS):
    if n_steps not in _NC_CACHE:
        _NC_CACHE[n_steps] = _build(n_steps)
    return _NC_CACHE[n_steps]


def _prep_in_maps(prop_state, A, in_W, in_b, out_W, out_b, r_W, r_b,
                  z_W, z_b, h_W, h_b, o1_W, o1_b, o2_W, o2_b):
    Af = np.ascontiguousarray(A).astype(np.float32)
    f32 = np.float32

    def rep(x):
        return np.ascontiguousarray(x, dtype=f32)

    def gate_w(W):
        # W.T is [3D, D]; chunk into [mo, 3D, 128] so each output tile's
        # weight block is one contiguous DMA
        return np.ascontiguousarray(W.T.astype(f32).reshape(3 * D, DT, 128).transpose(1, 0, 2))

    shared = {
        "in_wt": rep(in_W.T), "out_wt": rep(out_W.T),
        "r_wt": gate_w(r_W), "z_wt": gate_w(z_W), "h_wt": gate_w(h_W),
        "o1_wt": rep(o1_W.T), "o2_wt": rep(o2_W.T),
        "in_b_row": rep(in_b).reshape(1, D), "out_b_row": rep(out_b).reshape(1, D),
        "o2_b_row": rep(o2_b).reshape(1, D),
        "rb_half": rep(r_b / 2).reshape(DT, 128).T.copy(),
        "zb_half": rep(z_b / 2).reshape(DT, 128).T.copy(),
        "hb_col": rep(h_b).reshape(DT, 128).T.copy(),
        "o1b_col": rep(o1_b).reshape(DT, 128).T.copy(),
    }
    in_maps = []
    for k in range(NCORES):
        rows = slice(k * NS, (k + 1) * NS)
        m = dict(shared)
        m["state_t0"] = np.ascontiguousarray(prop_state[rows].astype(f32).T)
        m["a_in_rhs"] = np.ascontiguousarray(Af[rows, :].T.astype(np.float16))
        m["a_out_rhs"] = np.ascontiguousarray(Af[:, rows].astype(np.float16))
        in_maps.append(m)
    return in_maps


def run(trace=False, **inputs):
    nc = _get_nc()
    in_maps = _prep_in_maps(**inputs)
    res = bass_utils.run_bass_kernel_spmd(
        nc, in_maps, core_ids=list(range(NCORES)), trace=trace)
    out = np.concatenate([res.results[k]["out_shard"] for k in range(NCORES)], axis=0)
    return out, res


def kernel(**inputs) -> np.ndarray:
    out, _ = run(trace=False, **inputs)
    return out


# revision 25
# speedup vs baseline: 3.9140x; 1.8615x over previous
"""GGNN (gated graph NN) message-passing kernel for Trainium2, 8 NeuronCores.

Model (per reference):
  5 steps of: s_in = state @ in_W.T + in_b ; s_out = state @ out_W.T + out_b
              a_in = A @ s_in ; a_out = A.T @ s_out
              r = sigmoid([a_in, a_out, state] @ r_W.T + r_b)
              z = sigmoid([a_in, a_out, state] @ z_W.T + z_b)
              h = tanh([a_in, a_out, r*state] @ h_W.T + h_b)
              state = (1-z)*state + z*h
  out = tanh(state @ o1_W.T + o1_b) @ o2_W.T + o2_b

Sharding: 1D node parallelism over 8 cores (512 nodes each). Weights
replicated. Per step, each core computes its s_in/s_out shard, all-gathers
them, then computes its row-shard of the aggregations and gates locally.

Precision: the recurrence amplifies operand rounding ~2500x, so bf16 is
far too coarse. Aggregations (the big 4096-contraction GEMMs) use an
exact fp16 hi/lo split of s_in/s_out: A is 0/1 (exact in fp16), so
A@s = A@hi + (1/2048)*A@(2048*(s-hi)) recovers ~22 mantissa bits at
2 passes of full PE rate (2x faster than native fp32 matmul). The lo
part is pre-scaled by 2^11 to dodge fp16 subnormal flushing. All other
GEMMs run native fp32; sigmoid is computed as 0.5+0.5*tanh(x/2) (tanh
LUT is ~50x more accurate than the sigmoid LUT).
"""

import numpy as np

import concourse.bass as bass
import concourse.mybir as mybir
import concourse.tile as tile
from concourse import bacc
from concourse import bass_utils

N = 4096
D = 512
NCORES = 8
NS = N // NCORES          # 512 local nodes per core
NT = NS // 128            # 4 node tiles
DT = D // 128             # 4 feature tiles
CT = 3 * D // 128         # 12 concat-feature tiles
JT = N // 128             # 32 contraction tiles for aggregation
N_STEPS = 5
LO_SCALE = 2048.0         # 2^11: shift s-hi into fp16 normal range

F32 = mybir.dt.float32
F16 = mybir.dt.float16


def _build(n_steps=N_STEPS, use_collective=True):
    nc = bacc.Bacc("TRN2", target_bir_lowering=False, debug=False,
                   enable_asserts=True,
                   num_devices=NCORES if use_collective else 1)

    # ---- per-core external I/O ----
    state_t0 = nc.dram_tensor("state_t0", [D, NS], F32, kind="ExternalInput")
    a_in_rhs = nc.dram_tensor("a_in_rhs", [N, NS], F16, kind="ExternalInput")
    a_out_rhs = nc.dram_tensor("a_out_rhs", [N, NS], F16, kind="ExternalInput")
    in_wt = nc.dram_tensor("in_wt", [D, D], F32, kind="ExternalInput")
    out_wt = nc.dram_tensor("out_wt", [D, D], F32, kind="ExternalInput")
    # gate weights pre-chunked by output tile: [mo, kc*128, 128]
    r_wt = nc.dram_tensor("r_wt", [DT, 3 * D, 128], F32, kind="ExternalInput")
    z_wt = nc.dram_tensor("z_wt", [DT, 3 * D, 128], F32, kind="ExternalInput")
    h_wt = nc.dram_tensor("h_wt", [DT, 3 * D, 128], F32, kind="ExternalInput")
    o1_wt = nc.dram_tensor("o1_wt", [D, D], F32, kind="ExternalInput")
    o2_wt = nc.dram_tensor("o2_wt", [D, D], F32, kind="ExternalInput")
    in_b_row = nc.dram_tensor("in_b_row", [1, D], F32, kind="ExternalInput")
    out_b_row = nc.dram_tensor("out_b_row", [1, D], F32, kind="ExternalInput")
    o2_b_row = nc.dram_tensor("o2_b_row", [1, D], F32, kind="ExternalInput")
    rb_half = nc.dram_tensor("rb_half", [128, DT], F32, kind="ExternalInput")
    zb_half = nc.dram_tensor("zb_half", [128, DT], F32, kind="ExternalInput")
    hb_col = nc.dram_tensor("hb_col", [128, DT], F32, kind="ExternalInput")
    o1b_col = nc.dram_tensor("o1b_col", [128, DT], F32, kind="ExternalInput")
    out_shard = nc.dram_tensor("out_shard", [NS, D], F32, kind="ExternalOutput")

    with tile.TileContext(nc) as tc:
        with (
            tc.tile_pool(name="wpool", bufs=1) as wpool,
            tc.tile_pool(name="state", bufs=2) as spool,
            tc.tile_pool(name="gatew", bufs=3) as gwpool,
            tc.tile_pool(name="work", bufs=2) as work,
            tc.tile_pool(name="sjp", bufs=10) as sjp,
            tc.tile_pool(name="astp", bufs=10) as astp,
            tc.tile_pool(name="att", bufs=2) as att,
            tc.tile_pool(name="gout", bufs=1) as gout,
            tc.tile_pool(name="psum", bufs=8, space="PSUM") as psum,
            tc.tile_pool(name="dram", bufs=2, space="DRAM") as dram,
        ):
            # ---- resident weights / constants ----
            ain_sb = wpool.tile([128, JT, NS], F16, name="ain_sb")
            nc.sync.dma_start(ain_sb[:], a_in_rhs.ap().rearrange("(jt p) r -> p jt r", p=128))
            aout_dram = a_out_rhs.ap().rearrange("(jt p) r -> p jt r", p=128)
            inw_sb = wpool.tile([128, DT, D], F32, name="inw_sb")
            nc.sync.dma_start(inw_sb[:], in_wt.ap().rearrange("(ti p) o -> p ti o", p=128))
            outw_sb = wpool.tile([128, DT, D], F32, name="outw_sb")
            nc.sync.dma_start(outw_sb[:], out_wt.ap().rearrange("(ti p) o -> p ti o", p=128))
            inb_sb = wpool.tile([1, D], F32, name="inb_sb")
            nc.sync.dma_start(inb_sb[:], in_b_row.ap())
            outb_sb = wpool.tile([1, D], F32, name="outb_sb")
            nc.sync.dma_start(outb_sb[:], out_b_row.ap())
            o2b_sb = wpool.tile([1, D], F32, name="o2b_sb")
            nc.sync.dma_start(o2b_sb[:], o2_b_row.ap())
            rbh_sb = wpool.tile([128, DT], F32, name="rbh_sb")
            nc.sync.dma_start(rbh_sb[:], rb_half.ap())
            zbh_sb = wpool.tile([128, DT], F32, name="zbh_sb")
            nc.sync.dma_start(zbh_sb[:], zb_half.ap())
            hb_sb = wpool.tile([128, DT], F32, name="hb_sb")
            nc.sync.dma_start(hb_sb[:], hb_col.ap())
            o1b_sb = wpool.tile([128, DT], F32, name="o1b_sb")
            nc.sync.dma_start(o1b_sb[:], o1b_col.ap())
            ones_sb = wpool.tile([1, 128], F32, name="ones_sb")
            nc.vector.memset(ones_sb[:], 1.0)

            # ---- initial state (feature-major stateT [i, n]) ----
            st = spool.tile([128, DT, NS], F32, name="st")
            nc.sync.dma_start(st[:], state_t0.ap().rearrange("(ti p) n -> p ti n", p=128))

            for step in range(n_steps):
                in_cc_a = dram.tile([NS, 2 * D], F16, name="in_cc_a", tag="icc")
                in_cc_b = dram.tile([NS, 2 * D], F16, name="in_cc_b", tag="icc")
                # s_in gathered per 128-row block (4 small AllGathers) so the
                # first aggregation matmuls can start while GEMM1 is still
                # running; s_out as one gather (it hides under a_in compute).
                out_ccs = [dram.tile([8 * 128, 2 * D], F16, name=f"occ_a{nt}",
                                     tag="occa", addr_space="Shared")
                           for nt in range(NT)]
                out_cc_b = dram.tile([N, 2 * D], F16, name="out_cc_b", tag="occ",
                                     addr_space="Shared")

                # ---- GEMM1: s_in / s_out (node-major [n, o]) + bias, f16 hi/lo
                for w_sb, b_sb, icc, blockwise in ((inw_sb, inb_sb, in_cc_a, True),
                                                   (outw_sb, outb_sb, in_cc_b, False)):
                    for nt in range(NT):
                        ps = psum.tile([128, D], F32, name="ps", tag="ps")
                        for ti in range(DT):
                            nc.tensor.matmul(ps[:], st[:, ti, nt * 128:(nt + 1) * 128],
                                             w_sb[:, ti, :], start=(ti == 0), stop=False)
                        nc.tensor.matmul(ps[:], ones_sb[:], b_sb[:], start=False, stop=True)
                        hi = work.tile([128, D], F16, name="hi", tag="hi")
                        nc.vector.tensor_copy(hi[:], ps[:])
                        dd = work.tile([128, D], F32, name="dd", tag="dd")
                        nc.vector.tensor_sub(dd[:], ps[:], hi[:])
                        lo = work.tile([128, D], F16, name="lo", tag="lo")
                        nc.vector.tensor_scalar_mul(lo[:], dd[:], LO_SCALE)
                        nc.sync.dma_start(icc[nt * 128:(nt + 1) * 128, 0:D], hi[:])
                        nc.sync.dma_start(icc[nt * 128:(nt + 1) * 128, D:2 * D], lo[:])
                        if blockwise and use_collective:
                            nc.gpsimd.collective_compute(
                                "AllGather",
                                mybir.AluOpType.bypass,
                                replica_groups=[list(range(NCORES))],
                                ins=[icc[nt * 128:(nt + 1) * 128, :].opt()],
                                outs=[out_ccs[nt].opt()],
                            )
                    if not blockwise:
                        if use_collective:
                            nc.gpsimd.collective_compute(
                                "AllGather",
                                mybir.AluOpType.bypass,
                                replica_groups=[list(range(NCORES))],
                                ins=[icc.opt()],
                                outs=[out_cc_b.opt()],
                            )
                        else:
                            nc.sync.dma_start(out_cc_b[0:NS, :], icc[:])
                    elif not use_collective:
                        for nt in range(NT):
                            nc.sync.dma_start(out_ccs[nt][0:128, :],
                                              in_cc_a[nt * 128:(nt + 1) * 128, :])

                occ_b = out_cc_b.rearrange("(jt p) c -> p jt c", p=128)

                # ---- aggregations: a_inT / a_outT (feature-major [f, r]) ----
                a_inT = att.tile([128, DT, NS], F32, name="a_inT", tag="aT")
                a_outT = att.tile([128, DT, NS], F32, name="a_outT", tag="aT")

                # a_in: consume gather blocks in arrival order; block b core c
                # holds nodes j-tile c*NT + b
                ph = [psum.tile([128, NS], F32, name=f"ph{f}", tag="ps") for f in range(DT)]
                pl = [psum.tile([128, NS], F32, name=f"pl{f}", tag="ps") for f in range(DT)]
                for b in range(NT):
                    occ_bv = out_ccs[b].rearrange("(cc p) c -> p cc c", p=128)
                    for cc in range(NCORES):
                        jt = cc * NT + b
                        sj = sjp.tile([128, 2 * D], F16, name="sj", tag="sj")
                        nc.sync.dma_start(sj[:], occ_bv[:, cc, :])
                        first = (b == 0 and cc == 0)
                        last = (b == NT - 1 and cc == NCORES - 1)
                        for f in range(DT):
                            nc.tensor.matmul(ph[f][:], sj[:, f * 128:(f + 1) * 128],
                                             ain_sb[:, jt, :], start=first, stop=last)
                            nc.tensor.matmul(pl[f][:], sj[:, D + f * 128:D + (f + 1) * 128],
                                             ain_sb[:, jt, :], start=first, stop=last)
                for f in range(DT):
                    tmp = work.tile([128, NS], F32, name="tmph", tag="tmph")
                    nc.vector.tensor_copy(tmp[:], ph[f][:])
                    nc.vector.scalar_tensor_tensor(
                        a_inT[:, f, :], pl[f][:], 1.0 / LO_SCALE, tmp[:],
                        mybir.AluOpType.mult, mybir.AluOpType.add)

                # a_out
                ph = [psum.tile([128, NS], F32, name=f"qh{f}", tag="ps") for f in range(DT)]
                pl = [psum.tile([128, NS], F32, name=f"ql{f}", tag="ps") for f in range(DT)]
                for jt in range(JT):
                    sj = sjp.tile([128, 2 * D], F16, name="sj", tag="sj")
                    nc.sync.dma_start(sj[:], occ_b[:, jt, :])
                    ast = astp.tile([128, NS], F16, name="ast", tag="ast")
                    nc.sync.dma_start(ast[:], aout_dram[:, jt, :])
                    for f in range(DT):
                        nc.tensor.matmul(ph[f][:], sj[:, f * 128:(f + 1) * 128],
                                         ast[:], start=(jt == 0), stop=(jt == JT - 1))
                        nc.tensor.matmul(pl[f][:], sj[:, D + f * 128:D + (f + 1) * 128],
                                         ast[:], start=(jt == 0), stop=(jt == JT - 1))
                for f in range(DT):
                    tmp = work.tile([128, NS], F32, name="tmph", tag="tmph")
                    nc.vector.tensor_copy(tmp[:], ph[f][:])
                    nc.vector.scalar_tensor_tensor(
                        a_outT[:, f, :], pl[f][:], 1.0 / LO_SCALE, tmp[:],
                        mybir.AluOpType.mult, mybir.AluOpType.add)

                # ---- gates ----
                def a_rhs(c):
                    if c < DT:
                        return a_inT[:, c, :]
                    if c < 2 * DT:
                        return a_outT[:, c - DT, :]
                    return st[:, c - 2 * DT, :]

                rT = gout.tile([128, DT, NS], F32, name="rT", tag="rT")
                zT = gout.tile([128, DT, NS], F32, name="zT", tag="zT")

                def gate_pair(w_dram, rhs_fn, evict_fn, tilename):
                    for mo in range(DT):
                        gw = gwpool.tile([128, CT, 128], F32, name=tilename, tag="gw")
                        nc.sync.dma_start(
                            gw[:], w_dram.ap()[mo].rearrange("(kc p) m -> p kc m", p=128))
                        ps = psum.tile([128, NS], F32, name="psg", tag="ps")
                        for c in range(CT):
                            nc.tensor.matmul(ps[:], gw[:, c, :], rhs_fn(c),
                                             start=(c == 0), stop=(c == CT - 1))
                        evict_fn(mo, ps)

                def evict_r(mo, ps):
                    # T_r = tanh(0.5*pre + 0.5*b); r = 0.5 + 0.5*T_r
                    nc.scalar.activation(rT[:, mo, :], ps[:], mybir.ActivationFunctionType.Tanh,
                                         bias=rbh_sb[:, mo:mo + 1], scale=0.5)

                def evict_z(mo, ps):
                    nc.scalar.activation(zT[:, mo, :], ps[:], mybir.ActivationFunctionType.Tanh,
                                         bias=zbh_sb[:, mo:mo + 1], scale=0.5)

                gate_pair(r_wt, a_rhs, evict_r, "gw_r")
                gate_pair(z_wt, a_rhs, evict_z, "gw_z")

                # rs = (0.5 + 0.5*T_r) * state, written in place over rT
                for f in range(DT):
                    rfull = work.tile([128, NS], F32, name="rfull", tag="rfull")
                    nc.vector.tensor_scalar(rfull[:], rT[:, f, :], 0.5, 0.5,
                                            mybir.AluOpType.mult, mybir.AluOpType.add)
                    nc.vector.tensor_mul(rT[:, f, :], rfull[:], st[:, f, :])

                def j_rhs(c):
                    if c < 2 * DT:
                        return a_rhs(c)
                    return rT[:, c - 2 * DT, :]

                st_new = spool.tile([128, DT, NS], F32, name="st")

                def evict_h(mo, ps):
                    hh = work.tile([128, NS], F32, name="hh", tag="hh")
                    nc.scalar.activation(hh[:], ps[:], mybir.ActivationFunctionType.Tanh,
                                         bias=hb_sb[:, mo:mo + 1], scale=1.0)
                    # state' = state + (0.5 + 0.5*T_z)*(h - state)
                    d2 = work.tile([128, NS], F32, name="d2", tag="d2")
                    nc.vector.tensor_sub(d2[:], hh[:], st[:, mo, :])
                    e2 = work.tile([128, NS], F32, name="e2", tag="e2")
                    nc.vector.tensor_mul(e2[:], zT[:, mo, :], d2[:])
                    g2 = work.tile([128, NS], F32, name="g2", tag="g2")
                    nc.vector.tensor_add(g2[:], d2[:], e2[:])
                    nc.vector.scalar_tensor_tensor(
                        st_new[:, mo, :], g2[:], 0.5, st[:, mo, :],
                        mybir.AluOpType.mult, mybir.AluOpType.add)

                gate_pair(h_wt, j_rhs, evict_h, "gw_h")
                st = st_new

            # ---- output MLP ----
            o1w_sb = gwpool.tile([128, DT, D], F32, name="o1w_sb", tag="gw")
            nc.sync.dma_start(o1w_sb[:], o1_wt.ap().rearrange("(ti p) o -> p ti o", p=128))
            o2w_sb = gwpool.tile([128, DT, D], F32, name="o2w_sb", tag="gw")
            nc.sync.dma_start(o2w_sb[:], o2_wt.ap().rearrange("(ti p) o -> p ti o", p=128))

            tT = gout.tile([128, DT, NS], F32, name="tT", tag="rT")
            pst = [psum.tile([128, NS], F32, name=f"ps_t{mo}", tag="ps") for mo in range(DT)]
            for ti in range(DT):
                for mo in range(DT):
                    nc.tensor.matmul(pst[mo][:], o1w_sb[:, ti, mo * 128:(mo + 1) * 128],
                                     st[:, ti, :], start=(ti == 0), stop=(ti == DT - 1))
            for mo in range(DT):
                nc.scalar.activation(tT[:, mo, :], pst[mo][:], mybir.ActivationFunctionType.Tanh,
                                     bias=o1b_sb[:, mo:mo + 1], scale=1.0)

            out_sb = gout.tile([128, NT, D], F32, name="out_sb", tag="zT")
            pso = [psum.tile([128, D], F32, name=f"ps_o{nt}", tag="ps") for nt in range(NT)]
            for c in range(DT):
                for nt in range(NT):
                    nc.tensor.matmul(pso[nt][:], tT[:, c, nt * 128:(nt + 1) * 128],
                                     o2w_sb[:, c, :], start=(c == 0), stop=False)
            for nt in range(NT):
                nc.tensor.matmul(pso[nt][:], ones_sb[:], o2b_sb[:], start=False, stop=True)
                nc.vector.tensor_copy(out_sb[:, nt, :], pso[nt][:])
            nc.sync.dma_start(out_shard.ap().rearrange("(nt p) o -> p nt o", p=128), out_sb[:])

    nc.compile()
    return nc


_NC_CACHE = {}


def _get_nc(n_steps=N_STEPS):
    if n_steps not in _NC_CACHE:
        _NC_CACHE[n_steps] = _build(n_steps)
    return _NC_CACHE[n_steps]


def _prep_in_maps(prop_state, A, in_W, in_b, out_W, out_b, r_W, r_b,
                  z_W, z_b, h_W, h_b, o1_W, o1_b, o2_W, o2_b):
    Af = np.ascontiguousarray(A).astype(np.float32)
    f32 = np.float32

    def rep(x):
        return np.ascontiguousarray(x, dtype=f32)

    def gate_w(W):
        # W.T is [3D, D]; chunk into [mo, 3D, 128] so each output tile's
        # weight block is one contiguous DMA
        return np.ascontiguousarray(W.T.astype(f32).reshape(3 * D, DT, 128).transpose(1, 0, 2))

    shared = {
        "in_wt": rep(in_W.T), "out_wt": rep(out_W.T),
        "r_wt": gate_w(r_W), "z_wt": gate_w(z_W), "h_wt": gate_w(h_W),
        "o1_wt": rep(o1_W.T), "o2_wt": rep(o2_W.T),
        "in_b_row": rep(in_b).reshape(1, D), "out_b_row": rep(out_b).reshape(1, D),
        "o2_b_row": rep(o2_b).reshape(1, D),
        "rb_half": rep(r_b / 2).reshape(DT, 128).T.copy(),
        "zb_half": rep(z_b / 2).reshape(DT, 128).T.copy(),
        "hb_col": rep(h_b).reshape(DT, 128).T.copy(),
        "o1b_col": rep(o1_b).reshape(DT, 128).T.copy(),
    }
    in_maps = []
    for k in range(NCORES):
        rows = slice(k * NS, (k + 1) * NS)
        m = dict(shared)
        m["state_t0"] = np.ascontiguousarray(prop_state[rows].astype(f32).T)
        m["a_in_rhs"] = np.ascontiguousarray(Af[rows, :].T.astype(np.float16))
        m["a_out_rhs"] = np.ascontiguousarray(Af[:, rows].astype(np.float16))
        in_maps.append(m)
    return in_maps


def run(trace=False, **inputs):
    nc = _get_nc()
    in_maps = _prep_in_maps(**inputs)
    res = bass_utils.run_bass_kernel_spmd(
        nc, in_maps, core_ids=list(range(NCORES)), trace=trace)
    out = np.concatenate([res.results[k]["out_shard"] for k in range(NCORES)], axis=0)
    return out, res


def kernel(**inputs) -> np.ndarray:
    out, _ = run(trace=False, **inputs)
    return out
